# revision 1
# baseline (speedup 1.0000x reference)
"""Trainium2 Bass kernel for per-sample weight-demodulated 3x3 conv + leaky ReLU.

Problem (hardcoded shapes):
  input_vector: (8, 256, 128, 128) f32
  style_vector: (8, 256) f32
  weight:       (256, 256, 3, 3) f32
  out:          (8, 256, 128, 128) f32

Math (faithful to reference):
  ws[b,o,i,kh,kw] = weight[o,i,kh,kw] * style[b,i]
  demod[b,kw]     = rsqrt(sum_{o,i,kh} ws^2 + 1e-8)        # NOTE: sum excludes kw
  y[b] = leaky_relu(conv2d_same(x[b], ws[b]*demod), 0.2)

Sharding: data-parallel over batch, one sample per NeuronCore (8 cores).

Per-core kernel strategy:
  - Load weight as [cout_part, (cin,tap)] contiguous; PE-transpose 36 tiles of
    [128,128] into [cin_part, cout] per (tap, cin_blk, cout_blk); scale by
    style (per-partition) on the PSUM->SBUF copy.
  - Sum-of-squares per kw via DVE tensor_tensor_reduce per partition, then a
    ones[128,128] matmul to reduce across partitions (broadcast to all
    partitions for free); demod = 1/sqrt(. + 1e-8); rescale the 36 tiles.
  - x staged in SBUF as zero-padded 130-wide rows (130x130 per channel),
    split into 4 h-quarters so DMA overlaps compute.
  - Conv: per 4-row output chunk (N=512), accumulate 2cin_blk x 9tap x 2cout_blk
    matmuls in PSUM (fp32r: full PE rate at N>=512), leaky-relu on copy out.

`stage` (debug bisection): "wprep" dumps style-scaled transposed weights,
"demod" additionally runs the demod chain and rescale, "xio" dumps the staged
padded input quarters 0/3, "conv1" runs a single output chunk, "full" is the
real kernel.
"""

import numpy as np

B, CIN, COUT, K, H, W = 8, 256, 256, 3, 128, 128
P = 128
KB = CIN // P   # cin partition blocks   = 2
MB = COUT // P  # cout partition blocks  = 2
T = K * K       # taps = 9
WP = W + 2      # padded row width = 130
NQ = 4          # h-quarters
QROWS = H // NQ           # output rows per quarter = 32
QPAD = QROWS + 2          # padded rows held per quarter = 34
CHUNK_ROWS = 4            # output rows per psum chunk
CHUNK_N = CHUNK_ROWS * W  # matmul free size = 512

_CACHE = {}


def _build(stage="full"):
    import concourse.mybir as mybir
    import concourse.tile as tile
    from concourse import bacc
    from concourse.masks import make_identity

    f32 = mybir.dt.float32
    f32r = mybir.dt.float32r

    nc = bacc.Bacc(None, target_bir_lowering=False)
    x_d = nc.dram_tensor("x", [CIN, H, W], f32r, kind="ExternalInput")
    s_d = nc.dram_tensor("style", [1, CIN], f32, kind="ExternalInput")
    w_d = nc.dram_tensor("w", [COUT, CIN, K, K], f32, kind="ExternalInput")
    y_d = nc.dram_tensor("y", [COUT, H, W], f32, kind="ExternalOutput")

    x_flat = x_d[:].rearrange("c h w -> c (h w)")      # [256, 16384]
    y_flat = y_d[:].rearrange("o h w -> o (h w)")      # [256, 16384]
    w_flat = w_d[:].rearrange("o i kh kw -> o (i kh kw)")  # [256, 2304]

    with tile.TileContext(nc) as tc:
        with (
            tc.tile_pool(name="const", bufs=1) as const,
            tc.tile_pool(name="wtmp", bufs=1) as wtmp,
            tc.tile_pool(name="xbuf", bufs=1) as xbuf,
            tc.tile_pool(name="outp", bufs=3) as outp,
            tc.tile_pool(name="psum", bufs=2, space="PSUM") as psum,
            tc.tile_pool(name="psumw", bufs=2, space="PSUM") as psumw,
            tc.tile_pool(name="psumd", bufs=1, space="PSUM") as psumd,
        ):
            # ---------- constants ----------
            ident = const.tile([P, P], f32)
            make_identity(nc, ident)
            ones = const.tile([P, P], f32)
            nc.vector.memset(ones, 1.0)

            # style per-partition: stile[p, kb] = style[kb*128 + p]
            stile = const.tile([P, KB], f32)
            for kb in range(KB):
                nc.sync.dma_start(
                    out=stile[:, kb : kb + 1],
                    in_=s_d[:].rearrange("one c -> c one")[kb * P : (kb + 1) * P, :],
                )

            # ---------- raw weight load: wbuf[o_part, mb, (i,kh,kw)] ----------
            wbuf = wtmp.tile([P, MB, CIN * T], f32)
            nc.sync.dma_start(
                out=wbuf[:],
                in_=w_flat.rearrange("(m p) f -> p m f", p=P),
            )

            # ---------- transpose to [i_part, ...] and scale by style ----------
            # wfin[i_part, kb, t, mb, o] ; t = kh*3+kw
            wfin = const.tile([P, KB, T, MB, P], f32r)
            wview = wbuf[:].rearrange("p m (i t) -> p m t i", t=T)  # strided view
            # PE wait-slot pre-gating: the transpose-mode matmul ISA slot fits
            # only ONE semaphore wait; make the PE observe the identity's
            # (gpsimd) tick on a throwaway transpose so real transposes only
            # carry the weight-DMA wait.
            gate = psumd.tile([P, P], f32, name="gate")
            nc.tensor.transpose(gate, ident, ident)
            for kb in range(KB):
                for t in range(T):
                    for mb in range(MB):
                        pt = psumw.tile([P, P], f32)
                        nc.tensor.transpose(
                            pt, wview[:, mb, t, kb * P : (kb + 1) * P], ident
                        )
                        # wstyle = transposed * style_i (per-partition scalar)
                        nc.vector.tensor_scalar_mul(
                            out=wfin[:, kb, t, mb, :],
                            in0=pt,
                            scalar1=stile[:, kb : kb + 1],
                        )

            if stage == "wprep":
                ot = outp.tile([P, KB * T * MB * P], f32)
                nc.vector.tensor_copy(out=ot, in_=wfin[:].bitcast(f32))
                nc.sync.dma_start(out=y_flat[0:P, 0 : KB * T * MB * P], in_=ot)

            if stage not in ("wprep",):
                # ---------- demod: D[kw] = sum_{o,i,kh} wstyle^2 ----------
                sp = wtmp.tile([P, KB, K], f32)
                junk = wtmp.tile([P, K * MB * P], f32)
                for kb in range(KB):
                    for kw in range(K):
                        src = wfin[:, kb, kw::K, :, :].bitcast(f32)
                        jv = junk[:].rearrange("p (a b c) -> p a b c", a=K, b=MB)
                        nc.vector.tensor_tensor(
                            out=jv, in0=src, in1=src, op=mybir.AluOpType.mult
                        )
                        nc.vector.reduce_sum(
                            out=sp[:, kb, kw : kw + 1],
                            in_=jv,
                            axis=mybir.AxisListType.XYZ,
                        )
                # combine cin-block partials in SBUF; ones.T @ spc = cross-
                # partition sum broadcast to every partition
                spc = wtmp.tile([P, K], f32)
                nc.vector.tensor_add(out=spc, in0=sp[:, 0, :], in1=sp[:, 1, :])
                dps = psumd.tile([P, K], f32)
                nc.tensor.matmul(dps, ones, spc, start=True, stop=True)
                # demod = 1/sqrt(sum + 1e-8)
                demod = const.tile([P, K], f32)
                nc.vector.tensor_scalar_add(demod, dps, 1e-8)
                nc.scalar.sqrt(demod, demod)
                nc.vector.reciprocal(demod, demod)

                # rescale the 36 weight tiles in place by demod[kw]
                for kb in range(KB):
                    for t in range(T):
                        kw = t % K
                        nc.vector.tensor_scalar_mul(
                            out=wfin[:, kb, t, :, :],
                            in0=wfin[:, kb, t, :, :].bitcast(f32),
                            scalar1=demod[:, kw : kw + 1],
                        )

            if stage == "demod":
                ot = outp.tile([P, KB * T * MB * P], f32)
                nc.vector.tensor_copy(out=ot, in_=wfin[:].bitcast(f32))
                nc.sync.dma_start(out=y_flat[0:P, 0 : KB * T * MB * P], in_=ot)
                ot2 = outp.tile([P, K], f32)
                nc.vector.tensor_copy(out=ot2, in_=demod)
                nc.sync.dma_start(out=y_flat[0:P, 16000 : 16000 + K], in_=ot2)

            if stage not in ("wprep", "demod"):
                # ---------- input staging: 4 quarters, zero-padded ----------
                # memset can't target f32r; copy-cast zeros from an f32 tile
                zrow = const.tile([P, WP], f32)
                nc.vector.memset(zrow, 0.0)
                xqs = []
                for q in range(NQ):
                    xq = xbuf.tile([P, KB, QPAD, WP], f32r, name=f"xq{q}")
                    xqs.append(xq)
                    for kb in range(KB):
                        nc.vector.tensor_copy(out=xq[:, kb, :, 0], in_=zrow[:, :QPAD])
                        nc.vector.tensor_copy(
                            out=xq[:, kb, :, WP - 1], in_=zrow[:, :QPAD]
                        )
                        # rows of this quarter in image coords: 32q-1 .. 32q+32
                        img_lo = q * QROWS - 1
                        img_hi = q * QROWS + QROWS  # inclusive
                        lo_clip = max(img_lo, 0)
                        hi_clip = min(img_hi, H - 1)
                        if img_lo < 0:  # top zero row (local row 0)
                            nc.vector.tensor_copy(out=xq[:, kb, 0, :], in_=zrow)
                        if img_hi > H - 1:  # bottom zero row (local row QPAD-1)
                            nc.vector.tensor_copy(out=xq[:, kb, QPAD - 1, :], in_=zrow)
                        l_lo = lo_clip - img_lo  # local row of first valid img row
                        nrows = hi_clip - lo_clip + 1
                        nc.sync.dma_start(
                            out=xq[:, kb, l_lo : l_lo + nrows, 1 : 1 + W],
                            in_=x_flat[kb * P : (kb + 1) * P, :].rearrange(
                                "c (h w) -> c h w", w=W
                            )[:, lo_clip : hi_clip + 1, :],
                        )

                if stage == "xio":
                    for q in (0, 3):
                        for kb in range(KB):
                            ot = outp.tile([P, QPAD * WP], f32, name="xdump")
                            nc.vector.tensor_copy(
                                out=ot, in_=xqs[q][:, kb].bitcast(f32)
                            )
                            nc.sync.dma_start(
                                out=y_flat[
                                    kb * P : (kb + 1) * P,
                                    (q // 3) * 8000 : (q // 3) * 8000 + QPAD * WP,
                                ],
                                in_=ot,
                            )
                else:
                    # ---------- main conv loop ----------
                    nchunks = 1 if stage == "conv1" else H // CHUNK_ROWS
                    for c in range(nchunks):
                        r0 = c * CHUNK_ROWS
                        q = r0 // QROWS
                        lr0 = r0 - q * QROWS
                        xq = xqs[q]
                        pts = [
                            psum.tile([P, CHUNK_N], f32, name=f"pc{mb}")
                            for mb in range(MB)
                        ]
                        first = True
                        for kb in range(KB):
                            for t in range(T):
                                kh, kw = divmod(t, K)
                                rhs = xq[
                                    :, kb, lr0 + kh : lr0 + kh + CHUNK_ROWS,
                                    kw : kw + W,
                                ]
                                last = kb == KB - 1 and t == T - 1
                                for mb in range(MB):
                                    nc.tensor.matmul(
                                        pts[mb],
                                        wfin[:, kb, t, mb, :],
                                        rhs,
                                        start=first,
                                        stop=last,
                                    )
                                first = False
                        # leaky relu + store
                        for mb in range(MB):
                            ot = outp.tile([P, CHUNK_N], f32, name=f"ot{mb}")
                            tmp = outp.tile([P, CHUNK_N], f32, name=f"lt{mb}")
                            nc.vector.tensor_scalar_mul(tmp, pts[mb], 0.2)
                            nc.vector.tensor_tensor(
                                out=ot, in0=pts[mb], in1=tmp,
                                op=mybir.AluOpType.max,
                            )
                            nc.sync.dma_start(
                                out=y_flat[
                                    mb * P : (mb + 1) * P, r0 * W : r0 * W + CHUNK_N
                                ],
                                in_=ot,
                            )
    nc.compile()
    return nc


def _get_nc():
    if "nc" not in _CACHE:
        _CACHE["nc"] = _build()
    return _CACHE["nc"]


def kernel(input_vector, style_vector, weight):
    from concourse.bass_utils import run_bass_kernel_spmd

    input_vector = np.ascontiguousarray(input_vector, dtype=np.float32)
    style_vector = np.ascontiguousarray(style_vector, dtype=np.float32)
    weight = np.ascontiguousarray(weight, dtype=np.float32)

    nc = _get_nc()
    in_maps = [
        {
            "x": input_vector[b],
            "style": style_vector[b : b + 1],
            "w": weight,
        }
        for b in range(B)
    ]
    res = run_bass_kernel_spmd(nc, in_maps, core_ids=list(range(B)))
    out = np.stack([res.results[b]["y"] for b in range(B)], axis=0)
    return out

